# revision 15
# baseline (speedup 1.0000x reference)
"""GATv2Stack Trainium2 kernel (8-core data-parallel over graphs).

bt=128 graphs of N=64 nodes, 16 graphs/core. See reference.py.
  h = x @ W_in + b_in
  2x: xl=h@Wl+bl; xr=h@Wr+br; e=att.lrelu(xr_i+xl_j); a=softmax_j(e+mask)
      g = a@(h@Wl) + (out_bias+bl); g=ELU(g); g=LN(g); h=g+h; h=mask*h
  out = where(keep_graph, h, x@W_in+b_in)

Per-core layouts (G=16 graphs, gp=g//2, par=g%2):
  hT[m]      [128,1024] fp32r  [m*128+p, g*64+v] = h[g,v,m*128+p]
  h_node_w   [128,2048] fp32   [par*64+v, gp*256+d]
  xlT/xrT[m] [128,1024] fp32   (with bias)
  xl_node_w  [128,2048] fp32r  h@Wl (no bias; softmax rows sum to 1 so bl
                               folds into out_bias)
  sl (g,hp)  [128,4096] fp32r  lrelu(xr_i+xl_j), cols i*64+j
  e-mm: 4 streams (par,hp) -> shared psum [128,512] col-groups 32*s
  e_sb       [128,4096] fp32   stream-packed rows {32s+t}
  et_w       [128,2048] fp32   [par*64+i, gp*256+h*64+j]
  madd_w     [128,512]  additive mask; mvec_w [128,8] node mask
"""
import sys
sys.path.insert(0, '/opt/trn_rl_repo')
import numpy as np

import concourse.bass as bass
import concourse.mybir as mybir
from concourse import bass_utils, bacc
from concourse.tile import TileContext

dt = mybir.dt
F32, F32R = dt.float32, dt.float32r
AF = mybir.ActivationFunctionType
ALU = mybir.AluOpType

B, T, N, D_IN = 2, 64, 64, 512
HID, L, H, C = 256, 2, 4, 64
BT = B * T
G = 16
NCORES = 8
NEG_SLOPE = 0.2
LN_EPS = 1e-5
NEG_BIG = -1e9

_n = [0]
def _nm(p="t"):
    _n[0] += 1
    return f"{p}{_n[0]}"


def fd(ap, *dims):
    """Keep partition dim + offset of (sliced) AP, replace free dims."""
    return bass.AP(ap.tensor, ap.offset, [list(ap.ap[0])] + [[s, c] for (s, c) in dims])


def build_nc():
    nc = bacc.Bacc("TRN2", target_bir_lowering=False, debug=False,
                   enable_asserts=False, num_devices=1)

    def din(name, shape):
        return nc.dram_tensor(name, list(shape), F32, kind="ExternalInput").ap()

    x_d     = din("x_sh", [G * 64, D_IN])
    win_d   = din("w_in", [D_IN, HID])
    wl_d    = din("wl", [L, HID, HID])
    wr_d    = din("wr", [L, HID, HID])
    binT_d  = din("binT", [128, 2])
    blT_d   = din("blT", [128, 2 * L])
    brT_d   = din("brT", [128, 2 * L])
    obT_d   = din("obT", [128, 2 * L])
    att_d   = din("attBD", [128, 2 * 2 * L])
    gam_d   = din("gam_f", [L, 128, HID])
    bet_d   = din("bet_f", [L, 128, HID])
    swap_d  = din("swap_m", [128, 128])
    madd_d  = din("madd_w", [128, 8 * 64])
    mvec_d  = din("mvec_w", [128, 8])
    ident_d = din("ident", [128, 128])
    out_d   = nc.dram_tensor("out", [G * 64, HID], F32, kind="ExternalOutput").ap()

    with TileContext(nc) as tc:
        with tc.tile_pool(name="const", bufs=1) as cpool, \
             tc.tile_pool(name="wide", bufs=1) as wpool, \
             tc.tile_pool(name="slp", bufs=2) as slpool, \
             tc.tile_pool(name="sm", bufs=2) as smpool, \
             tc.tile_pool(name="psum", bufs=1, space="PSUM") as ppool:

            def ctile(name, dram_ap, shape, conv_r=False):
                t0 = cpool.tile(shape, F32, name=_nm(name))
                nc.sync.dma_start(t0[:], dram_ap)
                return t0

            win_r = win_d.rearrange("(k p) n -> k p n", p=128)
            win = [ctile(f"win{k}", win_r[k], [128, HID], True) for k in range(4)]
            wl, wr = [], []
            for l in range(L):
                wl_r = wl_d[l].rearrange("(k p) n -> k p n", p=128)
                wr_r = wr_d[l].rearrange("(k p) n -> k p n", p=128)
                wl.append([ctile(f"wl{l}{k}", wl_r[k], [128, HID], True) for k in range(2)])
                wr.append([ctile(f"wr{l}{k}", wr_r[k], [128, HID], True) for k in range(2)])
            binT  = ctile("binT", binT_d, [128, 2])
            blT   = ctile("blT", blT_d, [128, 2 * L])
            brT   = ctile("brT", brT_d, [128, 2 * L])
            obT   = ctile("obT", obT_d, [128, 2 * L])
            attBD = ctile("attBD", att_d, [128, 2 * 2 * L], True)
            gam   = [ctile(f"gam{l}", gam_d[l], [128, HID]) for l in range(L)]
            bet   = [ctile(f"bet{l}", bet_d[l], [128, HID]) for l in range(L)]
            swapm = ctile("swapm", swap_d, [128, 128])
            madd  = ctile("madd", madd_d, [128, 8 * 64])
            mvec  = ctile("mvec", mvec_d, [128, 8])
            ident = ctile("ident", ident_d, [128, 128])

            # ---------- input: load x, transpose, project ----------
            hT = [smpool.tile([128, G * 64], F32, name=_nm("hT"), tag=f"hT{m}", bufs=1)
                  for m in range(2)]
            h_node_w = smpool.tile([128, 8 * HID], F32, name=_nm("hnode"), tag="hnode", bufs=2)

            with tc.tile_pool(name="xtp", bufs=1) as xtpool:
                xT = [xtpool.tile([128, G * 64], F32, name=_nm("xT")) for _ in range(4)]
                x_rows = x_d.rearrange("(t p) d -> t p d", p=128)
                for t in range(8):
                    xrow = smpool.tile([128, D_IN], F32, name=_nm("xrow"), tag="xrow", bufs=3)
                    nc.sync.dma_start(xrow[:], x_rows[t])
                    for k in range(4):
                        pt = ppool.tile([128, 128], F32, name=_nm("pxt"), tag="tps", bufs=2)
                        nc.tensor.transpose(pt[:], xrow[:, k * 128:(k + 1) * 128], ident[:])
                        nc.vector.tensor_copy(xT[k][:, t * 128:(t + 1) * 128], pt[:])
                for m in range(2):
                    for cb in range(2):
                        ph = ppool.tile([128, 512], F32, name=_nm("ph"), tag="pps", bufs=2)
                        for k in range(4):
                            nc.tensor.matmul(ph[:], win[k][:, m * 128:(m + 1) * 128],
                                             xT[k][:, cb * 512:(cb + 1) * 512],
                                             start=(k == 0), stop=(k == 3))
                        nc.vector.tensor_scalar(hT[m][:, cb * 512:(cb + 1) * 512], ph[:],
                                                binT[:, m:m + 1], None, op0=ALU.add)
                for gp in range(8):
                    for m in range(2):
                        pt = ppool.tile([128, 128], F32, name=_nm("pnt"), tag="tps", bufs=2)
                        nc.tensor.transpose(
                            pt[:], hT[m][:, gp * 128:(gp + 1) * 128], ident[:])
                        nc.vector.tensor_scalar(
                            h_node_w[:, gp * HID + m * 128:gp * HID + m * 128 + 128],
                            pt[:], 1.0, None, op0=ALU.mult)

            # ---------- layers ----------
            for l in range(L):
                xlT = [smpool.tile([128, G * 64], F32, name=_nm("xlT"), tag=f"xlT{m}", bufs=1)
                       for m in range(2)]
                xrT = [smpool.tile([128, G * 64], F32, name=_nm("xrT"), tag=f"xrT{m}", bufs=1)
                       for m in range(2)]
                for (W, Tt, bvec) in ((wl[l], xlT, blT), (wr[l], xrT, brT)):
                    for m in range(2):
                        for cb in range(2):
                            pp = ppool.tile([128, 512], F32, name=_nm("pp"), tag="pps", bufs=2)
                            for k in range(2):
                                nc.tensor.matmul(pp[:], W[k][:, m * 128:(m + 1) * 128],
                                                 hT[k][:, cb * 512:(cb + 1) * 512],
                                                 start=(k == 0), stop=(k == 1))
                            nc.vector.tensor_scalar(Tt[m][:, cb * 512:(cb + 1) * 512], pp[:],
                                                    bvec[:, l * 2 + m:l * 2 + m + 1], None,
                                                    op0=ALU.add)
                # two pair-orders: A rows par*64 hold graph 2gp+par; B swapped
                xl_nodes = [smpool.tile([128, 8 * HID], F32, name=_nm("xlnode"),
                                        tag=f"xlnode{o}", bufs=1) for o in range(2)]
                for gp in range(8):
                    for m in range(2):
                        pt = ppool.tile([128, 128], F32, name=_nm("pxl"), tag="tps", bufs=2)
                        nc.tensor.transpose(
                            pt[:], xlT[m][:, gp * 128:(gp + 1) * 128], ident[:])
                        nc.vector.tensor_scalar(
                            xl_nodes[0][:, gp * HID + m * 128:gp * HID + m * 128 + 128],
                            pt[:], 1.0, None, op0=ALU.mult)
                for cb in range(4):
                    ps = ppool.tile([128, 512], F32, name=_nm("psw"), tag="pps", bufs=2)
                    nc.tensor.matmul(ps[:], swapm[:],
                                     xl_nodes[0][:, cb * 512:(cb + 1) * 512],
                                     start=True, stop=True)
                    nc.vector.tensor_scalar(xl_nodes[1][:, cb * 512:(cb + 1) * 512],
                                            ps[:], 1.0, None, op0=ALU.mult)

                # ---- attention: e rounds (one graph-pair = 4 streams/round) ----
                et_w = wpool.tile([128, 8 * HID], F32, name=_nm("etw"), tag="etw")
                for gp in range(8):
                    for half in range(2):  # i in [half*32, half*32+32)
                        sls = {}
                        for par in range(2):
                            g = gp * 2 + par
                            for hp in range(2):
                                sl = slpool.tile([128, 32 * 64], F32, name=_nm("sl"),
                                                 tag="sl", bufs=6)
                                xr_sl = xrT[hp][:, g * 64 + half * 32:g * 64 + half * 32 + 1]
                                xl_sl = xlT[hp][:, g * 64:g * 64 + 1]
                                nc.vector.tensor_tensor(sl[:],
                                                        fd(xr_sl, (1, 32), (0, 64)),
                                                        fd(xl_sl, (0, 32), (1, 64)),
                                                        op=ALU.add)
                                nc.scalar.activation(sl[:], sl[:], AF.Prelu, alpha=NEG_SLOPE)
                                sls[(par, hp)] = sl
                        e_sb = wpool.tile([128, 32 * 64], F32, name=_nm("esb"), tag="esb")
                        for ci in range(4):
                            pe = ppool.tile([128, 512], F32, name=_nm("pe"), tag="eps", bufs=2)
                            for par in range(2):
                                for hp in range(2):
                                    s_idx = par * 2 + hp
                                    nc.tensor.matmul(
                                        pe[32 * s_idx:32 * s_idx + 2, :],
                                        attBD[:, (l * 2 + hp) * 2:(l * 2 + hp) * 2 + 2],
                                        sls[(par, hp)][:, ci * 512:(ci + 1) * 512],
                                        start=True, stop=True,
                                        tile_position=(0, 32 * s_idx))
                            if ci % 2 == 0:
                                nc.vector.tensor_scalar(e_sb[:, ci * 512:(ci + 1) * 512],
                                                        pe[:], 1.0, None, op0=ALU.mult)
                            else:
                                nc.scalar.copy(e_sb[:, ci * 512:(ci + 1) * 512], pe[:])
                        # unpack: one DMA per (stream, head-in-pair)
                        for par in range(2):
                            for hp in range(2):
                                s_idx = par * 2 + hp
                                for t in range(2):
                                    h_g = 2 * hp + t
                                    src = fd(e_sb[32 * s_idx + t:32 * s_idx + t + 1, 0:1],
                                             (64, 32), (1, 64))
                                    dst_base = et_w[par * 64 + half * 32:
                                                    par * 64 + half * 32 + 32,
                                                    gp * HID + h_g * 64:
                                                    gp * HID + h_g * 64 + 1]
                                    dst = fd(dst_base, (1, 64))
                                    nc.sync.dma_start(dst, src)

                # ---- softmax over j (wide, in place) ----
                nc.vector.tensor_tensor(et_w[:], et_w[:],
                                        fd(madd[0:128, 0:1], (64, 8), (0, 4), (1, 64)),
                                        op=ALU.add)
                nc.scalar.activation(et_w[:], et_w[:], AF.Exp)
                z_w = smpool.tile([128, 32], F32, name=_nm("zw"), tag="zw", bufs=2)
                nc.vector.tensor_reduce(z_w[:],
                                        fd(et_w[0:128, 0:1], (256, 8), (64, 4), (1, 64)),
                                        axis=mybir.AxisListType.X, op=ALU.add)
                rz_w = smpool.tile([128, 32], F32, name=_nm("rzw"), tag="rzw", bufs=2)
                nc.vector.reciprocal(rz_w[:], z_w[:])
                nc.vector.tensor_tensor(et_w[:], et_w[:],
                                        fd(rz_w[0:128, 0:1], (4, 8), (1, 4), (0, 64)),
                                        op=ALU.mult)

                # ---- alpha transpose + out matmul (pair-batched, base-aligned) ----
                outT_w = [wpool.tile([128, G * 64], F32, name=_nm("outT"), tag=f"outT{hp}")
                          for hp in range(2)]
                for gp in range(8):
                    for hp in range(2):
                        # in [128(par,i), 128(t,j)] -> out [128(t,j), 128(par,i)]
                        pat2 = ppool.tile([128, 128], F32, name=_nm("pat"), tag="tps", bufs=2)
                        nc.tensor.transpose(
                            pat2[:], et_w[:, gp * HID + hp * 128:gp * HID + hp * 128 + 128],
                            ident[:])
                        aT2 = smpool.tile([128, 128], F32, name=_nm("aT"), tag="aT", bufs=3)
                        nc.vector.tensor_scalar(aT2[:], pat2[:], 1.0, None, op0=ALU.mult)
                        for par in range(2):
                            g = gp * 2 + par
                            po = ppool.tile([128, 64], F32, name=_nm("po"), tag="ops", bufs=2)
                            for t in range(2):
                                h_g = 2 * hp + t
                                xn = xl_nodes[0] if par == t else xl_nodes[1]
                                nc.tensor.matmul(
                                    po[t * 64:(t + 1) * 64, :],
                                    xn[t * 64:t * 64 + 64,
                                       gp * HID + h_g * 64:gp * HID + h_g * 64 + 64],
                                    aT2[t * 64:t * 64 + 64, par * 64:par * 64 + 64],
                                    start=True, stop=True)
                            nc.vector.tensor_scalar(outT_w[hp][:, g * 64:(g + 1) * 64],
                                                    po[:],
                                                    obT[:, l * 2 + hp:l * 2 + hp + 1], None,
                                                    op0=ALU.add)
                # ---- ELU (in place on outT_w) ----
                gelu_w = outT_w
                for hp in range(2):
                    tmin = smpool.tile([128, G * 64], F32, name=_nm("tmin"), tag="tmin", bufs=2)
                    nc.vector.tensor_scalar(tmin[:], outT_w[hp][:], 0.0, None, op0=ALU.min)
                    nc.scalar.activation(tmin[:], tmin[:], AF.Exp)
                    nc.vector.tensor_scalar(outT_w[hp][:], outT_w[hp][:], 0.0, None,
                                            op0=ALU.max)
                    nc.vector.scalar_tensor_tensor(outT_w[hp][:], outT_w[hp][:], -1.0,
                                                   tmin[:], op0=ALU.add, op1=ALU.add)
                # ---- transpose to node layout (full 128x128 blocks) ----
                gn_w = wpool.tile([128, 8 * HID], F32, name=_nm("gnw"), tag="gnw")
                for gp in range(8):
                    for hp in range(2):
                        pg2 = ppool.tile([128, 128], F32, name=_nm("pg"), tag="tps", bufs=2)
                        nc.tensor.transpose(pg2[:],
                                            gelu_w[hp][:, gp * 128:(gp + 1) * 128],
                                            ident[:])
                        nc.vector.tensor_scalar(
                            gn_w[:, gp * HID + hp * 128:gp * HID + hp * 128 + 128],
                            pg2[:], 1.0, None, op0=ALU.mult)

                # ---- LayerNorm + residual + mask ----
                sum_w = smpool.tile([128, 8], F32, name=_nm("sumw"), tag="sumw", bufs=2)
                nc.vector.tensor_reduce(sum_w[:],
                                        fd(gn_w[0:128, 0:1], (HID, 8), (1, HID)),
                                        axis=mybir.AxisListType.X, op=ALU.add)
                mu_w = smpool.tile([128, 8], F32, name=_nm("muw"), tag="muw", bufs=2)
                nc.vector.tensor_scalar(mu_w[:], sum_w[:], 1.0 / HID, None, op0=ALU.mult)
                nc.vector.tensor_tensor(gn_w[:], gn_w[:],
                                        fd(mu_w[0:128, 0:1], (1, 8), (0, HID)),
                                        op=ALU.subtract)
                cen2 = wpool.tile([128, 8 * HID], F32, name=_nm("cen2"), tag="esb")
                nc.vector.tensor_tensor(cen2[:], gn_w[:], gn_w[:], op=ALU.mult)
                vs_w = smpool.tile([128, 8], F32, name=_nm("vsw"), tag="vsw", bufs=2)
                nc.vector.tensor_reduce(vs_w[:],
                                        fd(cen2[0:128, 0:1], (HID, 8), (1, HID)),
                                        axis=mybir.AxisListType.X, op=ALU.add)
                var_w = smpool.tile([128, 8], F32, name=_nm("varw"), tag="varw", bufs=2)
                nc.vector.tensor_scalar(var_w[:], vs_w[:], 1.0 / HID, LN_EPS,
                                        op0=ALU.mult, op1=ALU.add)
                nc.scalar.activation(var_w[:], var_w[:], AF.Sqrt)
                rstd_w = smpool.tile([128, 8], F32, name=_nm("rstdw"), tag="rstdw", bufs=2)
                nc.vector.reciprocal(rstd_w[:], var_w[:])
                nc.vector.tensor_tensor(gn_w[:], gn_w[:],
                                        fd(rstd_w[0:128, 0:1], (1, 8), (0, HID)),
                                        op=ALU.mult)
                nc.vector.tensor_tensor(gn_w[:], gn_w[:],
                                        fd(gam[l][0:128, 0:1], (0, 8), (1, HID)),
                                        op=ALU.mult)
                nc.vector.tensor_tensor(gn_w[:], gn_w[:],
                                        fd(bet[l][0:128, 0:1], (0, 8), (1, HID)),
                                        op=ALU.add)
                hn_w = smpool.tile([128, 8 * HID], F32, name=_nm("hn"), tag="hnode", bufs=2)
                nc.vector.tensor_tensor(hn_w[:], gn_w[:], h_node_w[:], op=ALU.add)
                nc.vector.tensor_tensor(hn_w[:], hn_w[:],
                                        fd(mvec[0:128, 0:1], (1, 8), (0, HID)),
                                        op=ALU.mult)
                h_node_w = hn_w

                # ---- next-layer hT ----
                if l + 1 < L:
                    hT = [smpool.tile([128, G * 64], F32, name=_nm("hT"), tag=f"hT{m}",
                                      bufs=1) for m in range(2)]
                    for gp in range(8):
                        for m in range(2):
                            pt2 = ppool.tile([128, 128], F32, name=_nm("pht"), tag="tps",
                                             bufs=2)
                            nc.tensor.transpose(
                                pt2[:], hn_w[:, gp * HID + m * 128:gp * HID + m * 128 + 128],
                                ident[:])
                            nc.vector.tensor_scalar(hT[m][:, gp * 128:(gp + 1) * 128],
                                                    pt2[:], 1.0, None, op0=ALU.mult)

            # ---------- output DMA ----------
            for par in range(2):
                src = fd(h_node_w[par * 64:par * 64 + 64, 0:1], (HID, 8), (1, HID))
                dst_sl = out_d[par * 64:par * 64 + 1, :]
                dst = bass.AP(dst_sl.tensor, dst_sl.offset,
                              [[HID, 64], [2 * 64 * HID, 8], [1, HID]])
                nc.sync.dma_start(dst, src)

    nc.finalize()
    return nc


_CACHE = {}

def _get_nc():
    if "nc" not in _CACHE:
        _CACHE["nc"] = build_nc()
    return _CACHE["nc"]


def _host_prep(x, person_mask, W_in, b_in, Wl, bl, Wr, br, att, out_bias, ln_scale, ln_bias):
    x = np.asarray(x, np.float32).reshape(BT, N, D_IN)
    m = np.asarray(person_mask).reshape(BT, N)
    W_in = np.ascontiguousarray(np.asarray(W_in, np.float32))
    b_in = np.asarray(b_in, np.float32)
    Wl = np.ascontiguousarray(np.asarray(Wl, np.float32))
    bl = np.asarray(bl, np.float32)
    Wr = np.ascontiguousarray(np.asarray(Wr, np.float32))
    br = np.asarray(br, np.float32)
    att = np.asarray(att, np.float32)
    out_bias = np.asarray(out_bias, np.float32)
    ln_scale = np.asarray(ln_scale, np.float32)
    ln_bias = np.asarray(ln_bias, np.float32)

    binT = np.zeros((128, 2), np.float32)
    for mm in range(2):
        binT[:, mm] = b_in[mm * 128:(mm + 1) * 128]
    blT = np.zeros((128, 2 * L), np.float32)
    brT = np.zeros((128, 2 * L), np.float32)
    obT = np.zeros((128, 2 * L), np.float32)
    for l in range(L):
        for mm in range(2):
            blT[:, l * 2 + mm] = bl[l, mm * 128:(mm + 1) * 128]
            brT[:, l * 2 + mm] = br[l, mm * 128:(mm + 1) * 128]
            obT[:, l * 2 + mm] = out_bias[l, mm * 128:(mm + 1) * 128]
    attBD = np.zeros((128, 2 * 2 * L), np.float32)
    for l in range(L):
        for hp in range(2):
            for t in range(2):
                attBD[t * 64:(t + 1) * 64, (l * 2 + hp) * 2 + t] = att[l, 2 * hp + t]
    gam_f = np.repeat(ln_scale[:, None, :], 128, 1).astype(np.float32).copy()
    bet_f = np.repeat(ln_bias[:, None, :], 128, 1).astype(np.float32).copy()
    ident = np.eye(128, dtype=np.float32)
    swap_m = np.zeros((128, 128), np.float32)
    swap_m[0:64, 64:128] = np.eye(64)
    swap_m[64:128, 0:64] = np.eye(64)

    in_maps = []
    for c in range(NCORES):
        gs = slice(c * G, (c + 1) * G)
        xg = x[gs]
        mg = m[gs]
        madd_w = np.zeros((128, 8 * 64), np.float32)
        mvec_w = np.zeros((128, 8), np.float32)
        for g in range(G):
            gp, par = g // 2, g % 2
            allowed = (mg[g][:, None] & mg[g][None, :]) | np.eye(N, dtype=bool)
            madd_w[par * 64:(par + 1) * 64, gp * 64:(gp + 1) * 64] = \
                np.where(allowed, 0.0, NEG_BIG)
            mvec_w[par * 64:(par + 1) * 64, gp] = mg[g].astype(np.float32)
        in_maps.append({
            "x_sh": np.ascontiguousarray(xg.reshape(G * 64, D_IN)),
            "w_in": W_in, "wl": Wl, "wr": Wr,
            "binT": binT, "blT": blT, "brT": brT, "obT": obT,
            "attBD": attBD, "gam_f": gam_f, "bet_f": bet_f, "swap_m": swap_m,
            "madd_w": madd_w, "mvec_w": mvec_w, "ident": ident,
        })
    return in_maps, x, m, W_in, b_in


def kernel(**inputs) -> np.ndarray:
    in_maps, x, m, W_in, b_in = _host_prep(**inputs)
    nc = _get_nc()
    res = bass_utils.run_bass_kernel_spmd(nc, in_maps, core_ids=list(range(NCORES)))
    out = np.concatenate([res.results[c]["out"].reshape(G, N, HID)
                          for c in range(NCORES)], 0)
    keep = m.sum(-1) > 1
    if not keep.all():
        for g in np.nonzero(~keep)[0]:
            out[g] = x[g] @ W_in + b_in
    return out.reshape(B, T, N, HID)


# revision 21
# speedup vs baseline: 1.1617x; 1.1617x over previous
"""GATv2Stack Trainium2 kernel (8-core data-parallel over graphs).

bt=128 graphs of N=64 nodes, 16 graphs/core. See reference.py.
  h = x @ W_in + b_in
  2x: xl=h@Wl+bl; xr=h@Wr+br; e=att.lrelu(xr_i+xl_j); a=softmax_j(e+mask)
      g = a@(h@Wl) + (out_bias+bl); g=ELU(g); g=LN(g); h=g+h; h=mask*h
  out = where(keep_graph, h, x@W_in+b_in)

Per-core layouts (G=16 graphs, gp=g//2, par=g%2):
  hT[m]      [128,1024] fp32r  [m*128+p, g*64+v] = h[g,v,m*128+p]
  h_node_w   [128,2048] fp32   [par*64+v, gp*256+d]
  xlT/xrT[m] [128,1024] fp32   (with bias)
  xl_node_w  [128,2048] fp32r  h@Wl (no bias; softmax rows sum to 1 so bl
                               folds into out_bias)
  sl (g,hp)  [128,4096] fp32r  lrelu(xr_i+xl_j), cols i*64+j
  e-mm: 4 streams (par,hp) -> shared psum [128,512] col-groups 32*s
  e_sb       [128,4096] fp32   stream-packed rows {32s+t}
  et_w       [128,2048] fp32   [par*64+i, gp*256+h*64+j]
  madd_w     [128,512]  additive mask; mvec_w [128,8] node mask
"""
import sys, os
sys.path.insert(0, '/opt/trn_rl_repo')
USE_F32R = os.environ.get("NO_F32R", "0") != "1"
USE_F16SM = os.environ.get("NO_F16SM", "0") != "1" 
import numpy as np

import concourse.bass as bass
import concourse.mybir as mybir
from concourse import bass_utils, bacc
from concourse.tile import TileContext

dt = mybir.dt
F32, F32R, F16 = dt.float32, dt.float32r, dt.float16
if not USE_F32R:
    F32R = F32
AF = mybir.ActivationFunctionType
ALU = mybir.AluOpType

B, T, N, D_IN = 2, 64, 64, 512
HID, L, H, C = 256, 2, 4, 64
BT = B * T
G = 16
NCORES = 8
NEG_SLOPE = 0.2
LN_EPS = 1e-5
NEG_BIG = -30000.0

_n = [0]
def _nm(p="t"):
    _n[0] += 1
    return f"{p}{_n[0]}"


def fd(ap, *dims):
    """Keep partition dim + offset of (sliced) AP, replace free dims."""
    return bass.AP(ap.tensor, ap.offset, [list(ap.ap[0])] + [[s, c] for (s, c) in dims])


def build_nc():
    nc = bacc.Bacc("TRN2", target_bir_lowering=False, debug=False,
                   enable_asserts=False, num_devices=1)

    def din(name, shape):
        return nc.dram_tensor(name, list(shape), F32, kind="ExternalInput").ap()

    x_d     = din("x_sh", [G * 64, D_IN])
    win_d   = din("w_in", [D_IN, HID])
    wl_d    = din("wl", [L, HID, HID])
    wr_d    = din("wr", [L, HID, HID])
    binT_d  = din("binT", [128, 2])
    blT_d   = din("blT", [128, 2 * L])
    brT_d   = din("brT", [128, 2 * L])
    obT_d   = din("obT", [128, 2 * L])
    att_d   = din("attBD", [128, 2 * 2 * L])
    gam_d   = din("gam_f", [L, 128, HID])
    bet_d   = din("bet_f", [L, 128, HID])
    swap_d  = din("swap_m", [128, 128])
    madd_d  = din("madd_w", [128, 8 * 64])
    mvec_d  = din("mvec_w", [128, 8])
    ident_d = din("ident", [128, 128])
    out_d   = nc.dram_tensor("out", [G * 64, HID], F32, kind="ExternalOutput").ap()

    with TileContext(nc) as tc:
        with tc.tile_pool(name="const", bufs=1) as cpool, \
             tc.tile_pool(name="wide", bufs=1) as wpool, \
             tc.tile_pool(name="slp", bufs=2) as slpool, \
             tc.tile_pool(name="sm", bufs=2) as smpool, \
             tc.tile_pool(name="psum", bufs=1, space="PSUM") as ppool:

            def ctile(name, dram_ap, shape, conv_r=False):
                t0 = cpool.tile(shape, F32, name=_nm(name), tag=("ctmp" if conv_r else ""))
                nc.sync.dma_start(t0[:], dram_ap)
                if not conv_r:
                    return t0
                t1 = cpool.tile(shape, F32R, name=_nm(name + "r"))
                nc.vector.tensor_copy(t1[:], t0[:])
                return t1

            win_r = win_d.rearrange("(k p) n -> k p n", p=128)
            win = [ctile(f"win{k}", win_r[k], [128, HID], True) for k in range(4)]
            wl, wr = [], []
            for l in range(L):
                wl_r = wl_d[l].rearrange("(k p) n -> k p n", p=128)
                wr_r = wr_d[l].rearrange("(k p) n -> k p n", p=128)
                wl.append([ctile(f"wl{l}{k}", wl_r[k], [128, HID], True) for k in range(2)])
                wr.append([ctile(f"wr{l}{k}", wr_r[k], [128, HID], True) for k in range(2)])
            binT  = ctile("binT", binT_d, [128, 2])
            blT   = ctile("blT", blT_d, [128, 2 * L])
            brT   = ctile("brT", brT_d, [128, 2 * L])
            obT   = ctile("obT", obT_d, [128, 2 * L])
            attBD_f32 = ctile("attBD", att_d, [128, 2 * 2 * L])
            attBD08 = cpool.tile([128, 2 * 2 * L], F16, name=_nm("att08"))
            nc.vector.tensor_scalar(attBD08[:], attBD_f32[:], 0.8, None, op0=ALU.mult)
            attBD10 = cpool.tile([128, 2 * 2 * L], F16, name=_nm("att10"))
            nc.vector.tensor_copy(attBD10[:], attBD_f32[:])
            gam   = [ctile(f"gam{l}", gam_d[l], [128, HID]) for l in range(L)]
            bet   = [ctile(f"bet{l}", bet_d[l], [128, HID]) for l in range(L)]
            swapm = ctile("swapm", swap_d, [128, 128])
            madd_f32 = ctile("madd", madd_d, [128, 8 * 64])
            madd16 = cpool.tile([128, 8 * 64], F16, name=_nm("madd16"))
            nc.vector.tensor_copy(madd16[:], madd_f32[:])
            mvec  = ctile("mvec", mvec_d, [128, 8])
            ident = ctile("ident", ident_d, [128, 128])
            ident16 = cpool.tile([128, 128], F16, name=_nm("ident16"))
            nc.vector.tensor_copy(ident16[:], ident[:])
            identr = cpool.tile([128, 128], F32R, name=_nm("identr"))
            nc.vector.tensor_copy(identr[:], ident[:])
            nbias = cpool.tile([128, 1], F32, name=_nm("nbias"))
            nc.vector.memset(nbias[:], -4.0)

            # ---------- input: load x, transpose, project ----------
            hT = [smpool.tile([128, G * 64], F32R, name=_nm("hT"), tag=f"hT{m}", bufs=1)
                  for m in range(2)]
            h_node_w = smpool.tile([128, 8 * HID], F32, name=_nm("hnode"), tag="hnode", bufs=2)

            with tc.tile_pool(name="xtp", bufs=1) as xtpool:
                xT = [xtpool.tile([128, G * 64], F32R, name=_nm("xT")) for _ in range(4)]
                x_rows = x_d.rearrange("(t p) d -> t p d", p=128)
                for t in range(8):
                    xrow = smpool.tile([128, D_IN], F32, name=_nm("xrow"), tag="xrow", bufs=3)
                    nc.sync.dma_start(xrow[:], x_rows[t])
                    for k in range(4):
                        pt = ppool.tile([128, 128], F32, name=_nm("pxt"), tag="tps", bufs=2)
                        nc.tensor.transpose(pt[:], xrow[:, k * 128:(k + 1) * 128], ident[:])
                        nc.vector.tensor_copy(xT[k][:, t * 128:(t + 1) * 128], pt[:])
                for m in range(2):
                    for cb in range(2):
                        ph = ppool.tile([128, 512], F32, name=_nm("ph"), tag="pps", bufs=1)
                        for k in range(4):
                            nc.tensor.matmul(ph[:], win[k][:, m * 128:(m + 1) * 128],
                                             xT[k][:, cb * 512:(cb + 1) * 512],
                                             start=(k == 0), stop=(k == 3))
                        nc.vector.tensor_scalar(hT[m][:, cb * 512:(cb + 1) * 512], ph[:],
                                                binT[:, m:m + 1], None, op0=ALU.add)
                for gp in range(8):
                    for m in range(2):
                        pt = ppool.tile([128, 128], F32R, name=_nm("pnt"), tag="tpsr", bufs=1)
                        nc.tensor.transpose(
                            pt[:], hT[m][:, gp * 128:(gp + 1) * 128], identr[:])
                        nc.vector.tensor_scalar(
                            h_node_w[:, gp * HID + m * 128:gp * HID + m * 128 + 128],
                            pt[:], 1.0, None, op0=ALU.mult)

            # ---------- layers ----------
            for l in range(L):
                xlT = [smpool.tile([128, G * 64], F32, name=_nm("xlT"), tag=f"xlT{m}", bufs=1)
                       for m in range(2)]
                xrT = [smpool.tile([128, G * 64], F32, name=_nm("xrT"), tag=f"xrT{m}", bufs=1)
                       for m in range(2)]
                for (W, Tt, bvec) in ((wl[l], xlT, blT), (wr[l], xrT, brT)):
                    for m in range(2):
                        for cb in range(2):
                            pp = ppool.tile([128, 512], F32, name=_nm("pp"), tag="pps", bufs=1)
                            for k in range(2):
                                nc.tensor.matmul(pp[:], W[k][:, m * 128:(m + 1) * 128],
                                                 hT[k][:, cb * 512:(cb + 1) * 512],
                                                 start=(k == 0), stop=(k == 1))
                            nc.vector.tensor_scalar(Tt[m][:, cb * 512:(cb + 1) * 512], pp[:],
                                                    bvec[:, l * 2 + m:l * 2 + m + 1], None,
                                                    op0=ALU.add)
                # two pair-orders: A rows par*64 hold graph 2gp+par; B swapped
                xl_nodes = [smpool.tile([128, 8 * HID], F32, name=_nm("xlnode"),
                                        tag=f"xlnode{o}", bufs=1) for o in range(2)]
                for gp in range(8):
                    for m in range(2):
                        pt = ppool.tile([128, 128], F32, name=_nm("pxl"), tag="tps", bufs=2)
                        nc.tensor.transpose(
                            pt[:], xlT[m][:, gp * 128:(gp + 1) * 128], ident[:])
                        nc.vector.tensor_scalar(
                            xl_nodes[0][:, gp * HID + m * 128:gp * HID + m * 128 + 128],
                            pt[:], 1.0, None, op0=ALU.mult)
                for cb in range(4):
                    ps = ppool.tile([128, 512], F32, name=_nm("psw"), tag="pps", bufs=1)
                    nc.tensor.matmul(ps[:], swapm[:],
                                     xl_nodes[0][:, cb * 512:(cb + 1) * 512],
                                     start=True, stop=True)
                    nc.vector.tensor_scalar(xl_nodes[1][:, cb * 512:(cb + 1) * 512],
                                            ps[:], 1.0, None, op0=ALU.mult)

                # ---- fp16 broadcast operands: xr, and -xl (negated for max-term) ----
                xrTb = [smpool.tile([128, G * 64], F16, name=_nm("xrTb"), tag=f"xrTb{m}",
                                    bufs=1) for m in range(2)]
                xlTn = [smpool.tile([128, G * 64], F16, name=_nm("xlTn"), tag=f"xlTn{m}",
                                    bufs=1) for m in range(2)]
                xlTb = [smpool.tile([128, G * 64], F16, name=_nm("xlTb"), tag=f"xlTb{m}",
                                    bufs=1) for m in range(2)]
                for m in range(2):
                    nc.vector.tensor_scalar(xrTb[m][:], xrT[m][:], 1.0, None, op0=ALU.mult)
                    nc.vector.tensor_scalar(xlTn[m][:], xlT[m][:], -1.0, None, op0=ALU.mult)
                    nc.vector.tensor_scalar(xlTb[m][:], xlT[m][:], 1.0, None, op0=ALU.mult)

                # ---- attention: e rounds (one graph-pair = 4 streams/round) ----
                # e' = 0.8*att.max(-xl_j, xr_i) + (att.xl)_j   [0.2*att.xr_i cancels]
                et_w = wpool.tile([128, 8 * HID], F16, name=_nm("etw"), tag="etw")
                for gp in range(8):
                    # axl for the 4 streams -> psum rows 32s+t, then SBUF
                    paxl = ppool.tile([128, 64], F32, name=_nm("paxl"), tag="ops", bufs=1)
                    for par in range(2):
                        g = gp * 2 + par
                        for hp in range(2):
                            s_idx = par * 2 + hp
                            nc.tensor.matmul(
                                paxl[32 * s_idx:32 * s_idx + 2, :],
                                attBD10[:, (l * 2 + hp) * 2:(l * 2 + hp) * 2 + 2],
                                xlTb[hp][:, g * 64:(g + 1) * 64],
                                start=True, stop=True,
                                tile_position=(0, 32 * s_idx))
                    waxl = smpool.tile([128, 64], F32, name=_nm("waxl"), tag="waxl", bufs=2)
                    nc.scalar.copy(waxl[:], paxl[:])
                    for half in range(2):  # i in [half*32, half*32+32)
                        sls = {}
                        for par in range(2):
                            g = gp * 2 + par
                            for hp in range(2):
                                sl = slpool.tile([128, 32 * 64], F16, name=_nm("sl"),
                                                 tag="sl", bufs=6)
                                xr_sl = xrTb[hp][:, g * 64 + half * 32:g * 64 + half * 32 + 1]
                                xl_sl = xlTn[hp][:, g * 64:g * 64 + 1]
                                nc.vector.tensor_tensor(sl[:],
                                                        fd(xr_sl, (1, 32), (0, 64)),
                                                        fd(xl_sl, (0, 32), (1, 64)),
                                                        op=ALU.max)
                                sls[(par, hp)] = sl
                        e_sb = wpool.tile([128, 32 * 64], F16, name=_nm("esb"), tag="esb")
                        for ci in range(4):
                            pe = ppool.tile([128, 512], F32, name=_nm("pe"), tag="eps", bufs=2)
                            for par in range(2):
                                for hp in range(2):
                                    s_idx = par * 2 + hp
                                    nc.tensor.matmul(
                                        pe[32 * s_idx:32 * s_idx + 2, :],
                                        attBD08[:, (l * 2 + hp) * 2:(l * 2 + hp) * 2 + 2],
                                        sls[(par, hp)][:, ci * 512:(ci + 1) * 512],
                                        start=True, stop=True,
                                        tile_position=(0, 32 * s_idx))
                            # evict + add axl_j (broadcast over i)
                            nc.vector.scalar_tensor_tensor(
                                e_sb[:, ci * 512:(ci + 1) * 512], pe[:], 1.0,
                                fd(waxl[0:128, 0:1], (0, 8), (1, 64)),
                                op0=ALU.mult, op1=ALU.add)
                        # unpack: one DMA per (stream, head-in-pair)
                        for par in range(2):
                            for hp in range(2):
                                s_idx = par * 2 + hp
                                for t in range(2):
                                    h_g = 2 * hp + t
                                    src2 = fd(e_sb[32 * s_idx + t:32 * s_idx + t + 1, 0:1],
                                              (64, 32), (1, 64))
                                    dst_base = et_w[par * 64 + half * 32:
                                                    par * 64 + half * 32 + 32,
                                                    gp * HID + h_g * 64:
                                                    gp * HID + h_g * 64 + 1]
                                    dst = fd(dst_base, (1, 64))
                                    nc.sync.dma_start(dst, src2)

                # ---- softmax over j (wide, in place, fp16) ----
                nc.vector.tensor_tensor(et_w[:], et_w[:],
                                        fd(madd16[0:128, 0:1], (64, 8), (0, 4), (1, 64)),
                                        op=ALU.add)
                nc.scalar.activation(et_w[:], et_w[:], AF.Exp, bias=nbias[:])
                z_w = smpool.tile([128, 32], F32, name=_nm("zw"), tag="zw", bufs=2)
                nc.vector.tensor_reduce(z_w[:],
                                        fd(et_w[0:128, 0:1], (256, 8), (64, 4), (1, 64)),
                                        axis=mybir.AxisListType.X, op=ALU.add)
                rz_w = smpool.tile([128, 32], F16, name=_nm("rzw"), tag="rzw", bufs=2)
                with nc.allow_low_precision(reason="softmax weights fp16 is plenty"):
                    nc.vector.reciprocal(rz_w[:], z_w[:])
                nc.vector.tensor_tensor(et_w[:], et_w[:],
                                        fd(rz_w[0:128, 0:1], (4, 8), (1, 4), (0, 64)),
                                        op=ALU.mult)

                # ---- alpha transpose + out matmul (pair-batched, base-aligned) ----
                outT_w = [wpool.tile([128, G * 64], F32, name=_nm("outT"), tag=f"outT{hp}")
                          for hp in range(2)]
                for gp in range(8):
                    for hp in range(2):
                        # in [128(par,i), 128(t,j)] -> out [128(t,j), 128(par,i)]
                        pat2 = ppool.tile([128, 128], F16, name=_nm("pat16"), tag="tps16", bufs=1)
                        nc.tensor.transpose(
                            pat2[:], et_w[:, gp * HID + hp * 128:gp * HID + hp * 128 + 128],
                            ident16[:])
                        aT2 = smpool.tile([128, 128], F32, name=_nm("aT"), tag="aT", bufs=3)
                        nc.vector.tensor_scalar(aT2[:], pat2[:], 1.0, None, op0=ALU.mult)
                        for par in range(2):
                            g = gp * 2 + par
                            po = ppool.tile([128, 64], F32, name=_nm("po"), tag="ops", bufs=1)
                            for t in range(2):
                                h_g = 2 * hp + t
                                xn = xl_nodes[0] if par == t else xl_nodes[1]
                                nc.tensor.matmul(
                                    po[t * 64:(t + 1) * 64, :],
                                    xn[t * 64:t * 64 + 64,
                                       gp * HID + h_g * 64:gp * HID + h_g * 64 + 64],
                                    aT2[t * 64:t * 64 + 64, par * 64:par * 64 + 64],
                                    start=True, stop=True)
                            nc.vector.tensor_scalar(outT_w[hp][:, g * 64:(g + 1) * 64],
                                                    po[:],
                                                    obT[:, l * 2 + hp:l * 2 + hp + 1], None,
                                                    op0=ALU.add)
                # ---- ELU (in place on outT_w) ----
                gelu_w = outT_w
                for hp in range(2):
                    tmin = smpool.tile([128, G * 64], F32, name=_nm("tmin"), tag="tmin", bufs=2)
                    nc.vector.tensor_scalar(tmin[:], outT_w[hp][:], 0.0, None, op0=ALU.min)
                    nc.scalar.activation(tmin[:], tmin[:], AF.Exp)
                    nc.vector.tensor_scalar(outT_w[hp][:], outT_w[hp][:], 0.0, None,
                                            op0=ALU.max)
                    nc.vector.scalar_tensor_tensor(outT_w[hp][:], outT_w[hp][:], -1.0,
                                                   tmin[:], op0=ALU.add, op1=ALU.add)
                # ---- transpose to node layout (full 128x128 blocks) ----
                gn_w = wpool.tile([128, 8 * HID], F32, name=_nm("gnw"), tag="gnw")
                for gp in range(8):
                    for hp in range(2):
                        pg2 = ppool.tile([128, 128], F32, name=_nm("pg"), tag="tps", bufs=2)
                        nc.tensor.transpose(pg2[:],
                                            gelu_w[hp][:, gp * 128:(gp + 1) * 128],
                                            ident[:])
                        nc.vector.tensor_scalar(
                            gn_w[:, gp * HID + hp * 128:gp * HID + hp * 128 + 128],
                            pg2[:], 1.0, None, op0=ALU.mult)

                # ---- LayerNorm + residual + mask ----
                sum_w = smpool.tile([128, 8], F32, name=_nm("sumw"), tag="sumw", bufs=2)
                nc.vector.tensor_reduce(sum_w[:],
                                        fd(gn_w[0:128, 0:1], (HID, 8), (1, HID)),
                                        axis=mybir.AxisListType.X, op=ALU.add)
                mu_w = smpool.tile([128, 8], F32, name=_nm("muw"), tag="muw", bufs=2)
                nc.vector.tensor_scalar(mu_w[:], sum_w[:], 1.0 / HID, None, op0=ALU.mult)
                nc.vector.tensor_tensor(gn_w[:], gn_w[:],
                                        fd(mu_w[0:128, 0:1], (1, 8), (0, HID)),
                                        op=ALU.subtract)
                cen2 = wpool.tile([128, 8 * HID], F32, name=_nm("cen2"), tag="esb")
                nc.vector.tensor_tensor(cen2[:], gn_w[:], gn_w[:], op=ALU.mult)
                vs_w = smpool.tile([128, 8], F32, name=_nm("vsw"), tag="vsw", bufs=2)
                nc.vector.tensor_reduce(vs_w[:],
                                        fd(cen2[0:128, 0:1], (HID, 8), (1, HID)),
                                        axis=mybir.AxisListType.X, op=ALU.add)
                var_w = smpool.tile([128, 8], F32, name=_nm("varw"), tag="varw", bufs=2)
                nc.vector.tensor_scalar(var_w[:], vs_w[:], 1.0 / HID, LN_EPS,
                                        op0=ALU.mult, op1=ALU.add)
                nc.scalar.activation(var_w[:], var_w[:], AF.Sqrt)
                rstd_w = smpool.tile([128, 8], F32, name=_nm("rstdw"), tag="rstdw", bufs=2)
                nc.vector.reciprocal(rstd_w[:], var_w[:])
                nc.vector.tensor_tensor(gn_w[:], gn_w[:],
                                        fd(rstd_w[0:128, 0:1], (1, 8), (0, HID)),
                                        op=ALU.mult)
                nc.vector.tensor_tensor(gn_w[:], gn_w[:],
                                        fd(gam[l][0:128, 0:1], (0, 8), (1, HID)),
                                        op=ALU.mult)
                nc.vector.tensor_tensor(gn_w[:], gn_w[:],
                                        fd(bet[l][0:128, 0:1], (0, 8), (1, HID)),
                                        op=ALU.add)
                hn_w = smpool.tile([128, 8 * HID], F32, name=_nm("hn"), tag="hnode", bufs=2)
                nc.vector.tensor_tensor(hn_w[:], gn_w[:], h_node_w[:], op=ALU.add)
                nc.vector.tensor_tensor(hn_w[:], hn_w[:],
                                        fd(mvec[0:128, 0:1], (1, 8), (0, HID)),
                                        op=ALU.mult)
                h_node_w = hn_w

                # ---- next-layer hT ----
                if l + 1 < L:
                    hT = [smpool.tile([128, G * 64], F32R, name=_nm("hT"), tag=f"hT{m}",
                                      bufs=1) for m in range(2)]
                    for gp in range(8):
                        for m in range(2):
                            pt2 = ppool.tile([128, 128], F32, name=_nm("pht"), tag="tps",
                                             bufs=2)
                            nc.tensor.transpose(
                                pt2[:], hn_w[:, gp * HID + m * 128:gp * HID + m * 128 + 128],
                                ident[:])
                            nc.vector.tensor_scalar(hT[m][:, gp * 128:(gp + 1) * 128],
                                                    pt2[:], 1.0, None, op0=ALU.mult)

            # ---------- output DMA ----------
            for par in range(2):
                src = fd(h_node_w[par * 64:par * 64 + 64, 0:1], (HID, 8), (1, HID))
                dst_sl = out_d[par * 64:par * 64 + 1, :]
                dst = bass.AP(dst_sl.tensor, dst_sl.offset,
                              [[HID, 64], [2 * 64 * HID, 8], [1, HID]])
                nc.sync.dma_start(dst, src)

    nc.finalize()
    return nc


_CACHE = {}

def _get_nc():
    if "nc" not in _CACHE:
        _CACHE["nc"] = build_nc()
    return _CACHE["nc"]


def _host_prep(x, person_mask, W_in, b_in, Wl, bl, Wr, br, att, out_bias, ln_scale, ln_bias):
    x = np.asarray(x, np.float32).reshape(BT, N, D_IN)
    m = np.asarray(person_mask).reshape(BT, N)
    W_in = np.ascontiguousarray(np.asarray(W_in, np.float32))
    b_in = np.asarray(b_in, np.float32)
    Wl = np.ascontiguousarray(np.asarray(Wl, np.float32))
    bl = np.asarray(bl, np.float32)
    Wr = np.ascontiguousarray(np.asarray(Wr, np.float32))
    br = np.asarray(br, np.float32)
    att = np.asarray(att, np.float32)
    out_bias = np.asarray(out_bias, np.float32)
    ln_scale = np.asarray(ln_scale, np.float32)
    ln_bias = np.asarray(ln_bias, np.float32)

    binT = np.zeros((128, 2), np.float32)
    for mm in range(2):
        binT[:, mm] = b_in[mm * 128:(mm + 1) * 128]
    blT = np.zeros((128, 2 * L), np.float32)
    brT = np.zeros((128, 2 * L), np.float32)
    obT = np.zeros((128, 2 * L), np.float32)
    for l in range(L):
        for mm in range(2):
            blT[:, l * 2 + mm] = bl[l, mm * 128:(mm + 1) * 128]
            brT[:, l * 2 + mm] = br[l, mm * 128:(mm + 1) * 128]
            obT[:, l * 2 + mm] = out_bias[l, mm * 128:(mm + 1) * 128]
    attBD = np.zeros((128, 2 * 2 * L), np.float32)
    for l in range(L):
        for hp in range(2):
            for t in range(2):
                attBD[t * 64:(t + 1) * 64, (l * 2 + hp) * 2 + t] = att[l, 2 * hp + t]
    gam_f = np.repeat(ln_scale[:, None, :], 128, 1).astype(np.float32).copy()
    bet_f = np.repeat(ln_bias[:, None, :], 128, 1).astype(np.float32).copy()
    ident = np.eye(128, dtype=np.float32)
    swap_m = np.zeros((128, 128), np.float32)
    swap_m[0:64, 64:128] = np.eye(64)
    swap_m[64:128, 0:64] = np.eye(64)

    in_maps = []
    for c in range(NCORES):
        gs = slice(c * G, (c + 1) * G)
        xg = x[gs]
        mg = m[gs]
        madd_w = np.zeros((128, 8 * 64), np.float32)
        mvec_w = np.zeros((128, 8), np.float32)
        for g in range(G):
            gp, par = g // 2, g % 2
            allowed = (mg[g][:, None] & mg[g][None, :]) | np.eye(N, dtype=bool)
            madd_w[par * 64:(par + 1) * 64, gp * 64:(gp + 1) * 64] = \
                np.where(allowed, 0.0, NEG_BIG)
            mvec_w[par * 64:(par + 1) * 64, gp] = mg[g].astype(np.float32)
        in_maps.append({
            "x_sh": np.ascontiguousarray(xg.reshape(G * 64, D_IN)),
            "w_in": W_in, "wl": Wl, "wr": Wr,
            "binT": binT, "blT": blT, "brT": brT, "obT": obT,
            "attBD": attBD, "gam_f": gam_f, "bet_f": bet_f, "swap_m": swap_m,
            "madd_w": madd_w, "mvec_w": mvec_w, "ident": ident,
        })
    return in_maps, x, m, W_in, b_in


def kernel(**inputs) -> np.ndarray:
    in_maps, x, m, W_in, b_in = _host_prep(**inputs)
    nc = _get_nc()
    res = bass_utils.run_bass_kernel_spmd(nc, in_maps, core_ids=list(range(NCORES)))
    out = np.concatenate([res.results[c]["out"].reshape(G, N, HID)
                          for c in range(NCORES)], 0)
    keep = m.sum(-1) > 1
    if not keep.all():
        for g in np.nonzero(~keep)[0]:
            out[g] = x[g] @ W_in + b_in
    return out.reshape(B, T, N, HID)


# revision 25
# speedup vs baseline: 1.5736x; 1.3545x over previous
"""GATv2Stack Trainium2 kernel (8-core data-parallel over graphs).

bt=128 graphs of N=64 nodes, 16 graphs/core. See reference.py.
  h = x @ W_in + b_in
  2x: xl=h@Wl+bl; xr=h@Wr+br; e=att.lrelu(xr_i+xl_j); a=softmax_j(e+mask)
      g = a@(h@Wl) + (out_bias+bl); g=ELU(g); g=LN(g); h=g+h; h=mask*h
  out = where(keep_graph, h, x@W_in+b_in)

Per-core layouts (G=16 graphs, gp=g//2, par=g%2):
  hT[m]      [128,1024] fp32r  [m*128+p, g*64+v] = h[g,v,m*128+p]
  h_node_w   [128,2048] fp32   [par*64+v, gp*256+d]
  xlT/xrT[m] [128,1024] fp32   (with bias)
  xl_node_w  [128,2048] fp32r  h@Wl (no bias; softmax rows sum to 1 so bl
                               folds into out_bias)
  sl (g,hp)  [128,4096] fp32r  lrelu(xr_i+xl_j), cols i*64+j
  e-mm: 4 streams (par,hp) -> shared psum [128,512] col-groups 32*s
  e_sb       [128,4096] fp32   stream-packed rows {32s+t}
  et_w       [128,2048] fp32   [par*64+i, gp*256+h*64+j]
  madd_w     [128,512]  additive mask; mvec_w [128,8] node mask
"""
import sys, os
sys.path.insert(0, '/opt/trn_rl_repo')
USE_F32R = os.environ.get("NO_F32R", "0") != "1"
USE_F16SM = os.environ.get("NO_F16SM", "0") != "1" 
import numpy as np

import concourse.bass as bass
import concourse.mybir as mybir
from concourse import bass_utils, bacc
from concourse.tile import TileContext

dt = mybir.dt
F32, F32R, F16 = dt.float32, dt.float32r, dt.float16
if not USE_F32R:
    F32R = F32
AF = mybir.ActivationFunctionType
ALU = mybir.AluOpType

B, T, N, D_IN = 2, 64, 64, 512
HID, L, H, C = 256, 2, 4, 64
BT = B * T
G = 16
NCORES = 8
NEG_SLOPE = 0.2
LN_EPS = 1e-5
NEG_BIG = -30000.0

_n = [0]
def _nm(p="t"):
    _n[0] += 1
    return f"{p}{_n[0]}"


def fd(ap, *dims):
    """Keep partition dim + offset of (sliced) AP, replace free dims."""
    return bass.AP(ap.tensor, ap.offset, [list(ap.ap[0])] + [[s, c] for (s, c) in dims])


def build_nc(mh=(64,) * G):
    nc = bacc.Bacc("TRN2", target_bir_lowering=False, debug=False,
                   enable_asserts=False, num_devices=1)

    def din(name, shape):
        return nc.dram_tensor(name, list(shape), F32, kind="ExternalInput").ap()

    x_d     = din("x_sh", [G * 64, D_IN])
    win_d   = din("w_in", [D_IN, HID])
    wl_d    = din("wl", [L, HID, HID])
    wr_d    = din("wr", [L, HID, HID])
    binT_d  = din("binT", [128, 2])
    blT_d   = din("blT", [128, 2 * L])
    brT_d   = din("brT", [128, 2 * L])
    obT_d   = din("obT", [128, 2 * L])
    att_d   = din("attBD", [128, 2 * 2 * L])
    gam_d   = din("gam_f", [L, 128, HID])
    bet_d   = din("bet_f", [L, 128, HID])
    swap_d  = din("swap_m", [128, 128])
    madd_d  = din("madd_w", [128, 8 * 64])
    mvec_d  = din("mvec_w", [128, 8])
    ident_d = din("ident", [128, 128])
    out_d   = nc.dram_tensor("out", [G * 64, HID], F32, kind="ExternalOutput").ap()

    with TileContext(nc) as tc:
        with tc.tile_pool(name="const", bufs=1) as cpool, \
             tc.tile_pool(name="wide", bufs=1) as wpool, \
             tc.tile_pool(name="slp", bufs=2) as slpool, \
             tc.tile_pool(name="sm", bufs=2) as smpool, \
             tc.tile_pool(name="psum", bufs=1, space="PSUM") as ppool:

            def ctile(name, dram_ap, shape, conv_r=False):
                t0 = cpool.tile(shape, F32, name=_nm(name), tag=("ctmp" if conv_r else ""))
                nc.sync.dma_start(t0[:], dram_ap)
                if not conv_r:
                    return t0
                t1 = cpool.tile(shape, F32R, name=_nm(name + "r"))
                nc.vector.tensor_copy(t1[:], t0[:])
                return t1

            win_r = win_d.rearrange("(k p) n -> k p n", p=128)
            win = [ctile(f"win{k}", win_r[k], [128, HID], True) for k in range(4)]
            wl, wr = [], []
            for l in range(L):
                wl_r = wl_d[l].rearrange("(k p) n -> k p n", p=128)
                wr_r = wr_d[l].rearrange("(k p) n -> k p n", p=128)
                wl.append([ctile(f"wl{l}{k}", wl_r[k], [128, HID], True) for k in range(2)])
                wr.append([ctile(f"wr{l}{k}", wr_r[k], [128, HID], True) for k in range(2)])
            binT  = ctile("binT", binT_d, [128, 2])
            blT   = ctile("blT", blT_d, [128, 2 * L])
            brT   = ctile("brT", brT_d, [128, 2 * L])
            obT   = ctile("obT", obT_d, [128, 2 * L])
            attBD_f32 = ctile("attBD", att_d, [128, 2 * 2 * L])
            attBD08 = cpool.tile([128, 2 * 2 * L], F16, name=_nm("att08"))
            nc.vector.tensor_scalar(attBD08[:], attBD_f32[:], 0.8, None, op0=ALU.mult)
            attBD10 = cpool.tile([128, 2 * 2 * L], F16, name=_nm("att10"))
            nc.vector.tensor_copy(attBD10[:], attBD_f32[:])
            gam   = [ctile(f"gam{l}", gam_d[l], [128, HID]) for l in range(L)]
            bet   = [ctile(f"bet{l}", bet_d[l], [128, HID]) for l in range(L)]
            swapm = ctile("swapm", swap_d, [128, 128])
            madd_f32 = ctile("madd", madd_d, [128, 8 * 64])
            madd16 = cpool.tile([128, 8 * 64], F16, name=_nm("madd16"))
            nc.vector.tensor_copy(madd16[:], madd_f32[:])
            mvec  = ctile("mvec", mvec_d, [128, 8])
            ident = ctile("ident", ident_d, [128, 128])
            ident16 = cpool.tile([128, 128], F16, name=_nm("ident16"))
            nc.vector.tensor_copy(ident16[:], ident[:])
            identr = cpool.tile([128, 128], F32R, name=_nm("identr"))
            nc.vector.tensor_copy(identr[:], ident[:])
            nbias = cpool.tile([128, 1], F32, name=_nm("nbias"))
            nc.vector.memset(nbias[:], -4.0)

            # ---------- input: load x, transpose, project ----------
            hT = [smpool.tile([128, G * 64], F32R, name=_nm("hT"), tag=f"hT{m}", bufs=1)
                  for m in range(2)]
            h_node_w = smpool.tile([128, 8 * HID], F32, name=_nm("hnode"), tag="hnode", bufs=2)

            with tc.tile_pool(name="xtp", bufs=1) as xtpool:
                xT = [xtpool.tile([128, G * 64], F32R, name=_nm("xT")) for _ in range(4)]
                x_rows = x_d.rearrange("(t p) d -> t p d", p=128)
                for t in range(8):
                    xrow = smpool.tile([128, D_IN], F32, name=_nm("xrow"), tag="xrow", bufs=3)
                    nc.sync.dma_start(xrow[:], x_rows[t])
                    for k in range(4):
                        pt = ppool.tile([128, 128], F32, name=_nm("pxt"), tag="tps", bufs=2)
                        nc.tensor.transpose(pt[:], xrow[:, k * 128:(k + 1) * 128], ident[:])
                        nc.vector.tensor_copy(xT[k][:, t * 128:(t + 1) * 128], pt[:])
                for m in range(2):
                    for cb in range(2):
                        ph = ppool.tile([128, 512], F32, name=_nm("ph"), tag="pps", bufs=1)
                        for k in range(4):
                            nc.tensor.matmul(ph[:], win[k][:, m * 128:(m + 1) * 128],
                                             xT[k][:, cb * 512:(cb + 1) * 512],
                                             start=(k == 0), stop=(k == 3))
                        nc.vector.tensor_scalar(hT[m][:, cb * 512:(cb + 1) * 512], ph[:],
                                                binT[:, m:m + 1], None, op0=ALU.add)
                for gp in range(8):
                    for m in range(2):
                        pt = ppool.tile([128, 128], F32R, name=_nm("pnt"), tag="tpsr", bufs=1)
                        nc.tensor.transpose(
                            pt[:], hT[m][:, gp * 128:(gp + 1) * 128], identr[:])
                        nc.vector.tensor_scalar(
                            h_node_w[:, gp * HID + m * 128:gp * HID + m * 128 + 128],
                            pt[:], 1.0, None, op0=ALU.mult)

            # ---------- layers ----------
            for l in range(L):
                xlT = [smpool.tile([128, G * 64], F32, name=_nm("xlT"), tag=f"xlT{m}", bufs=1)
                       for m in range(2)]
                xrT = [smpool.tile([128, G * 64], F32, name=_nm("xrT"), tag=f"xrT{m}", bufs=1)
                       for m in range(2)]
                for (W, Tt, bvec) in ((wl[l], xlT, blT), (wr[l], xrT, brT)):
                    for m in range(2):
                        for cb in range(2):
                            pp = ppool.tile([128, 512], F32, name=_nm("pp"), tag="pps", bufs=1)
                            for k in range(2):
                                nc.tensor.matmul(pp[:], W[k][:, m * 128:(m + 1) * 128],
                                                 hT[k][:, cb * 512:(cb + 1) * 512],
                                                 start=(k == 0), stop=(k == 1))
                            nc.vector.tensor_scalar(Tt[m][:, cb * 512:(cb + 1) * 512], pp[:],
                                                    bvec[:, l * 2 + m:l * 2 + m + 1], None,
                                                    op0=ALU.add)
                # two pair-orders: A rows par*64 hold graph 2gp+par; B swapped
                xl_nodes = [smpool.tile([128, 8 * HID], F32, name=_nm("xlnode"),
                                        tag=f"xlnode{o}", bufs=1) for o in range(2)]
                for gp in range(8):
                    for m in range(2):
                        pt = ppool.tile([128, 128], F32, name=_nm("pxl"), tag="tps", bufs=2)
                        nc.tensor.transpose(
                            pt[:], xlT[m][:, gp * 128:(gp + 1) * 128], ident[:])
                        nc.vector.tensor_scalar(
                            xl_nodes[0][:, gp * HID + m * 128:gp * HID + m * 128 + 128],
                            pt[:], 1.0, None, op0=ALU.mult)
                for cb in range(4):
                    ps = ppool.tile([128, 512], F32, name=_nm("psw"), tag="pps", bufs=1)
                    nc.tensor.matmul(ps[:], swapm[:],
                                     xl_nodes[0][:, cb * 512:(cb + 1) * 512],
                                     start=True, stop=True)
                    nc.vector.tensor_scalar(xl_nodes[1][:, cb * 512:(cb + 1) * 512],
                                            ps[:], 1.0, None, op0=ALU.mult)

                # ---- fp16 broadcast operands: xr, and -xl (negated for max-term) ----
                xrTb = [smpool.tile([128, G * 64], F16, name=_nm("xrTb"), tag=f"xrTb{m}",
                                    bufs=1) for m in range(2)]
                xlTn = [smpool.tile([128, G * 64], F16, name=_nm("xlTn"), tag=f"xlTn{m}",
                                    bufs=1) for m in range(2)]
                xlTb = [smpool.tile([128, G * 64], F16, name=_nm("xlTb"), tag=f"xlTb{m}",
                                    bufs=1) for m in range(2)]
                for m in range(2):
                    nc.vector.tensor_scalar(xrTb[m][:], xrT[m][:], 1.0, None, op0=ALU.mult)
                    nc.vector.tensor_scalar(xlTn[m][:], xlT[m][:], -1.0, None, op0=ALU.mult)
                    nc.vector.tensor_scalar(xlTb[m][:], xlT[m][:], 1.0, None, op0=ALU.mult)

                # ---- attention: e rounds (one graph-pair = 4 streams/round) ----
                # e' = 0.8*att.max(-xl_j, xr_i) + (att.xl)_j   [0.2*att.xr_i cancels]
                # mh is per-PAIR uniform: mh[2k] == mh[2k+1]
                et_w = wpool.tile([128, 8 * HID], F16, name=_nm("etw"), tag="etw")
                nc.gpsimd.memset(et_w[:], 0.0)
                for gp in range(8):
                    m = mh[2 * gp]
                    paxl = ppool.tile([128, 64], F32, name=_nm("paxl"), tag="ops", bufs=1)
                    for par in range(2):
                        g = gp * 2 + par
                        for hp in range(2):
                            s_idx = par * 2 + hp
                            nc.tensor.matmul(
                                paxl[32 * s_idx:32 * s_idx + 2, :],
                                attBD10[:, (l * 2 + hp) * 2:(l * 2 + hp) * 2 + 2],
                                xlTb[hp][:, g * 64:(g + 1) * 64],
                                start=True, stop=True,
                                tile_position=(0, 32 * s_idx))
                    waxl = smpool.tile([128, 64], F32, name=_nm("waxl"), tag="waxl", bufs=2)
                    nc.scalar.copy(waxl[:], paxl[:])
                    for half in range((m + 31) // 32):
                        i0 = half * 32
                        i_cnt = min(32, m - i0)
                        ipc = max(d for d in (1, 2, 4, 8, 16, 24, 32)
                                  if i_cnt % d == 0 and d * m <= 512)
                        nch = i_cnt // ipc
                        w = ipc * m
                        sls = {}
                        for par in range(2):
                            g = gp * 2 + par
                            for hp in range(2):
                                sl = slpool.tile([128, i_cnt * m], F16, name=_nm("sl"),
                                                 tag="sl", bufs=6,
                                                 padded_shape=[128, 32 * 64])
                                xr_sl = xrTb[hp][:, g * 64 + i0:g * 64 + i0 + 1]
                                xl_sl = xlTn[hp][:, g * 64:g * 64 + 1]
                                nc.vector.tensor_tensor(sl[:],
                                                        fd(xr_sl, (1, i_cnt), (0, m)),
                                                        fd(xl_sl, (0, i_cnt), (1, m)),
                                                        op=ALU.max)
                                sls[(par, hp)] = sl
                        e_sb = wpool.tile([128, 32 * 64], F16, name=_nm("esb"), tag="esb")
                        for ci in range(nch):
                            pe = ppool.tile([128, 512], F32, name=_nm("pe"), tag="eps",
                                            bufs=2)
                            for par in range(2):
                                for hp in range(2):
                                    s_idx = par * 2 + hp
                                    nc.tensor.matmul(
                                        pe[32 * s_idx:32 * s_idx + 2, 0:w],
                                        attBD08[:, (l * 2 + hp) * 2:(l * 2 + hp) * 2 + 2],
                                        sls[(par, hp)][:, ci * w:(ci + 1) * w],
                                        start=True, stop=True,
                                        tile_position=(0, 32 * s_idx))
                            nc.vector.scalar_tensor_tensor(
                                e_sb[:, ci * w:(ci + 1) * w], pe[:, 0:w], 1.0,
                                fd(waxl[0:128, 0:1], (0, ipc), (1, m)),
                                op0=ALU.mult, op1=ALU.add)
                        for par in range(2):
                            for hp in range(2):
                                s_idx = par * 2 + hp
                                for t in range(2):
                                    h_g = 2 * hp + t
                                    src2 = fd(e_sb[32 * s_idx + t:32 * s_idx + t + 1, 0:1],
                                              (m, i_cnt), (1, m))
                                    dst_base = et_w[par * 64 + i0:par * 64 + i0 + i_cnt,
                                                    gp * HID + h_g * 64:
                                                    gp * HID + h_g * 64 + 1]
                                    dst = fd(dst_base, (1, m))
                                    nc.sync.dma_start(dst, src2)

                # ---- softmax over j (wide, in place, fp16) ----
                nc.vector.tensor_tensor(et_w[:], et_w[:],
                                        fd(madd16[0:128, 0:1], (64, 8), (0, 4), (1, 64)),
                                        op=ALU.add)
                nc.scalar.activation(et_w[:], et_w[:], AF.Exp, bias=nbias[:])
                z_w = smpool.tile([128, 32], F32, name=_nm("zw"), tag="zw", bufs=2)
                nc.vector.tensor_reduce(z_w[:],
                                        fd(et_w[0:128, 0:1], (256, 8), (64, 4), (1, 64)),
                                        axis=mybir.AxisListType.X, op=ALU.add)
                rz_w = smpool.tile([128, 32], F16, name=_nm("rzw"), tag="rzw", bufs=2)
                with nc.allow_low_precision(reason="softmax weights fp16 is plenty"):
                    nc.vector.reciprocal(rz_w[:], z_w[:])
                nc.vector.tensor_tensor(et_w[:], et_w[:],
                                        fd(rz_w[0:128, 0:1], (4, 8), (1, 4), (0, 64)),
                                        op=ALU.mult)

                # ---- alpha transpose + out matmul (pair-batched, base-aligned) ----
                outT_w = [wpool.tile([128, G * 64], F32, name=_nm("outT"), tag=f"outT{hp}")
                          for hp in range(2)]
                for gp in range(8):
                    for hp in range(2):
                        # in [128(par,i), 128(t,j)] -> out [128(t,j), 128(par,i)]
                        pat2 = ppool.tile([128, 128], F16, name=_nm("pat16"), tag="tps16", bufs=1)
                        nc.tensor.transpose(
                            pat2[:], et_w[:, gp * HID + hp * 128:gp * HID + hp * 128 + 128],
                            ident16[:])
                        aT2 = smpool.tile([128, 128], F32, name=_nm("aT"), tag="aT", bufs=3)
                        nc.vector.tensor_scalar(aT2[:], pat2[:], 1.0, None, op0=ALU.mult)
                        for par in range(2):
                            g = gp * 2 + par
                            po = ppool.tile([128, 64], F32, name=_nm("po"), tag="ops", bufs=1)
                            for t in range(2):
                                h_g = 2 * hp + t
                                xn = xl_nodes[0] if par == t else xl_nodes[1]
                                nc.tensor.matmul(
                                    po[t * 64:(t + 1) * 64, :],
                                    xn[t * 64:t * 64 + 64,
                                       gp * HID + h_g * 64:gp * HID + h_g * 64 + 64],
                                    aT2[t * 64:t * 64 + 64, par * 64:par * 64 + 64],
                                    start=True, stop=True)
                            nc.vector.tensor_scalar(outT_w[hp][:, g * 64:(g + 1) * 64],
                                                    po[:],
                                                    obT[:, l * 2 + hp:l * 2 + hp + 1], None,
                                                    op0=ALU.add)
                # ---- ELU (in place on outT_w) ----
                gelu_w = outT_w
                for hp in range(2):
                    tmin = smpool.tile([128, G * 64], F32, name=_nm("tmin"), tag="tmin", bufs=2)
                    nc.vector.tensor_scalar(tmin[:], outT_w[hp][:], 0.0, None, op0=ALU.min)
                    nc.scalar.activation(tmin[:], tmin[:], AF.Exp)
                    nc.vector.tensor_scalar(outT_w[hp][:], outT_w[hp][:], 0.0, None,
                                            op0=ALU.max)
                    nc.vector.scalar_tensor_tensor(outT_w[hp][:], outT_w[hp][:], -1.0,
                                                   tmin[:], op0=ALU.add, op1=ALU.add)
                # ---- transpose to node layout (full 128x128 blocks) ----
                gn_w = wpool.tile([128, 8 * HID], F32, name=_nm("gnw"), tag="gnw")
                for gp in range(8):
                    for hp in range(2):
                        pg2 = ppool.tile([128, 128], F32, name=_nm("pg"), tag="tps", bufs=2)
                        nc.tensor.transpose(pg2[:],
                                            gelu_w[hp][:, gp * 128:(gp + 1) * 128],
                                            ident[:])
                        nc.vector.tensor_scalar(
                            gn_w[:, gp * HID + hp * 128:gp * HID + hp * 128 + 128],
                            pg2[:], 1.0, None, op0=ALU.mult)

                # ---- LayerNorm + residual + mask ----
                sum_w = smpool.tile([128, 8], F32, name=_nm("sumw"), tag="sumw", bufs=2)
                nc.vector.tensor_reduce(sum_w[:],
                                        fd(gn_w[0:128, 0:1], (HID, 8), (1, HID)),
                                        axis=mybir.AxisListType.X, op=ALU.add)
                mu_w = smpool.tile([128, 8], F32, name=_nm("muw"), tag="muw", bufs=2)
                nc.vector.tensor_scalar(mu_w[:], sum_w[:], 1.0 / HID, None, op0=ALU.mult)
                nc.vector.tensor_tensor(gn_w[:], gn_w[:],
                                        fd(mu_w[0:128, 0:1], (1, 8), (0, HID)),
                                        op=ALU.subtract)
                cen2 = wpool.tile([128, 8 * HID], F32, name=_nm("cen2"), tag="esb")
                nc.vector.tensor_tensor(cen2[:], gn_w[:], gn_w[:], op=ALU.mult)
                vs_w = smpool.tile([128, 8], F32, name=_nm("vsw"), tag="vsw", bufs=2)
                nc.vector.tensor_reduce(vs_w[:],
                                        fd(cen2[0:128, 0:1], (HID, 8), (1, HID)),
                                        axis=mybir.AxisListType.X, op=ALU.add)
                var_w = smpool.tile([128, 8], F32, name=_nm("varw"), tag="varw", bufs=2)
                nc.vector.tensor_scalar(var_w[:], vs_w[:], 1.0 / HID, LN_EPS,
                                        op0=ALU.mult, op1=ALU.add)
                nc.scalar.activation(var_w[:], var_w[:], AF.Sqrt)
                rstd_w = smpool.tile([128, 8], F32, name=_nm("rstdw"), tag="rstdw", bufs=2)
                nc.vector.reciprocal(rstd_w[:], var_w[:])
                nc.vector.tensor_tensor(gn_w[:], gn_w[:],
                                        fd(rstd_w[0:128, 0:1], (1, 8), (0, HID)),
                                        op=ALU.mult)
                nc.vector.tensor_tensor(gn_w[:], gn_w[:],
                                        fd(gam[l][0:128, 0:1], (0, 8), (1, HID)),
                                        op=ALU.mult)
                nc.vector.tensor_tensor(gn_w[:], gn_w[:],
                                        fd(bet[l][0:128, 0:1], (0, 8), (1, HID)),
                                        op=ALU.add)
                hn_w = smpool.tile([128, 8 * HID], F32, name=_nm("hn"), tag="hnode", bufs=2)
                nc.vector.tensor_tensor(hn_w[:], gn_w[:], h_node_w[:], op=ALU.add)
                nc.vector.tensor_tensor(hn_w[:], hn_w[:],
                                        fd(mvec[0:128, 0:1], (1, 8), (0, HID)),
                                        op=ALU.mult)
                h_node_w = hn_w

                # ---- next-layer hT ----
                if l + 1 < L:
                    hT = [smpool.tile([128, G * 64], F32R, name=_nm("hT"), tag=f"hT{m}",
                                      bufs=1) for m in range(2)]
                    for gp in range(8):
                        for m in range(2):
                            pt2 = ppool.tile([128, 128], F32, name=_nm("pht"), tag="tps",
                                             bufs=2)
                            nc.tensor.transpose(
                                pt2[:], hn_w[:, gp * HID + m * 128:gp * HID + m * 128 + 128],
                                ident[:])
                            nc.vector.tensor_scalar(hT[m][:, gp * 128:(gp + 1) * 128],
                                                    pt2[:], 1.0, None, op0=ALU.mult)

            # ---------- output DMA ----------
            for par in range(2):
                src = fd(h_node_w[par * 64:par * 64 + 64, 0:1], (HID, 8), (1, HID))
                dst_sl = out_d[par * 64:par * 64 + 1, :]
                dst = bass.AP(dst_sl.tensor, dst_sl.offset,
                              [[HID, 64], [2 * 64 * HID, 8], [1, HID]])
                nc.sync.dma_start(dst, src)

    nc.finalize()
    return nc


_CACHE = {}

def _get_nc(mh):
    mh = tuple(mh)
    if mh not in _CACHE:
        _CACHE[mh] = build_nc(mh)
    return _CACHE[mh]


def _host_prep(x, person_mask, W_in, b_in, Wl, bl, Wr, br, att, out_bias, ln_scale, ln_bias):
    x = np.asarray(x, np.float32).reshape(BT, N, D_IN)
    m = np.asarray(person_mask).reshape(BT, N)
    W_in = np.ascontiguousarray(np.asarray(W_in, np.float32))
    b_in = np.asarray(b_in, np.float32)
    Wl = np.ascontiguousarray(np.asarray(Wl, np.float32))
    bl = np.asarray(bl, np.float32)
    Wr = np.ascontiguousarray(np.asarray(Wr, np.float32))
    br = np.asarray(br, np.float32)
    att = np.asarray(att, np.float32)
    out_bias = np.asarray(out_bias, np.float32)
    ln_scale = np.asarray(ln_scale, np.float32)
    ln_bias = np.asarray(ln_bias, np.float32)

    # ---- pack active nodes; stripe sorted graphs across cores ----
    n_g = m.sum(-1).astype(np.int64)                     # active counts
    order = np.argsort(-n_g, kind="stable")              # desc
    idxs = [np.nonzero(m[g])[0] for g in range(BT)]
    mh = []
    for s in range(G):
        n_top = n_g[order[s * NCORES]]
        mh.append(max(8, int(-(-int(n_top) // 8) * 8)))
    for k in range(0, G, 2):                             # pair-equalize
        mh[k + 1] = mh[k]
    mh = tuple(min(64, v) for v in mh)

    binT = np.zeros((128, 2), np.float32)
    for mm in range(2):
        binT[:, mm] = b_in[mm * 128:(mm + 1) * 128]
    blT = np.zeros((128, 2 * L), np.float32)
    brT = np.zeros((128, 2 * L), np.float32)
    obT = np.zeros((128, 2 * L), np.float32)
    for l in range(L):
        for mm in range(2):
            blT[:, l * 2 + mm] = bl[l, mm * 128:(mm + 1) * 128]
            brT[:, l * 2 + mm] = br[l, mm * 128:(mm + 1) * 128]
            obT[:, l * 2 + mm] = out_bias[l, mm * 128:(mm + 1) * 128]
    attBD = np.zeros((128, 2 * 2 * L), np.float32)
    for l in range(L):
        for hp in range(2):
            for t in range(2):
                attBD[t * 64:(t + 1) * 64, (l * 2 + hp) * 2 + t] = att[l, 2 * hp + t]
    gam_f = np.repeat(ln_scale[:, None, :], 128, 1).astype(np.float32).copy()
    bet_f = np.repeat(ln_bias[:, None, :], 128, 1).astype(np.float32).copy()
    ident = np.eye(128, dtype=np.float32)
    swap_m = np.zeros((128, 128), np.float32)
    swap_m[0:64, 64:128] = np.eye(64)
    swap_m[64:128, 0:64] = np.eye(64)

    in_maps = []
    for c in range(NCORES):
        xg = np.zeros((G * 64, D_IN), np.float32)
        madd_w = np.full((128, 8 * 64), NEG_BIG, np.float32)
        mvec_w = np.zeros((128, 8), np.float32)
        for s in range(G):
            gg = order[s * NCORES + c]
            n = int(n_g[gg])
            gp, par = s // 2, s % 2
            if n > 0:
                xg[s * 64:s * 64 + n] = x[gg][idxs[gg]]
                madd_w[par * 64:(par + 1) * 64, gp * 64:gp * 64 + n] = 0.0
                mvec_w[par * 64 + np.arange(n), gp] = 1.0
            else:
                madd_w[par * 64:(par + 1) * 64, gp * 64] = 0.0
        in_maps.append({
            "x_sh": xg, "w_in": W_in, "wl": Wl, "wr": Wr,
            "binT": binT, "blT": blT, "brT": brT, "obT": obT,
            "attBD": attBD, "gam_f": gam_f, "bet_f": bet_f, "swap_m": swap_m,
            "madd_w": madd_w, "mvec_w": mvec_w, "ident": ident,
        })
    return in_maps, x, m, W_in, b_in, order, idxs, n_g, mh


def kernel(**inputs) -> np.ndarray:
    in_maps, x, m, W_in, b_in, order, idxs, n_g, mh = _host_prep(**inputs)
    nc = _get_nc(mh)
    res = bass_utils.run_bass_kernel_spmd(nc, in_maps, core_ids=list(range(NCORES)))
    out = np.zeros((BT, N, HID), np.float32)
    for c in range(NCORES):
        dev = res.results[c]["out"].reshape(G, 64, HID)
        for s in range(G):
            gg = order[s * NCORES + c]
            n = int(n_g[gg])
            if n > 0:
                out[gg][idxs[gg]] = dev[s, :n]
    keep = n_g > 1
    if not keep.all():
        for g in np.nonzero(~keep)[0]:
            out[g] = x[g] @ W_in + b_in
    return out.reshape(B, T, N, HID)
